# revision 1
# baseline (speedup 1.0000x reference)
"""Trainium2 Bass kernel v2 for nn_CoNe_82995948028056 (gnn_message_passing).

Self-contained. N=8192, D=32, Q=4096, C_IN=128, C=256, 8 cores.
vs v1: bf16 datapath, host-folded BN0/deg/bias rows, host-transposed x,
single-pass GCN, precomputed vT, chunked bf16 z AllGather with BN3 folded
after the gather, chunk-masked content gathers.
"""

import numpy as np

N, D, Q, C_IN, C = 8192, 32, 4096, 128, 256
EPS = 1e-5
NC = 8
NS = N // NC      # 1024 node shard
QS = Q // NC      # 512 query shard
P = 128
NT = N // P       # 64
QT = QS // P      # 4
CH = 4            # z AllGather chunks
BIG = 300.0
OOB = 1 << 20

_CACHE = {}


def _build(debug=False):
    import concourse.bacc as bacc
    import concourse.mybir as mybir
    import concourse.tile as tile
    import concourse.bass as bass
    from concourse.masks import make_identity

    dt = mybir.dt
    F32, F32R, BF16, I32 = dt.float32, dt.float32r, dt.bfloat16, dt.int32
    AF = mybir.ActivationFunctionType
    OP = mybir.AluOpType
    AX = mybir.AxisListType

    nc = bacc.Bacc("TRN2", target_bir_lowering=False)

    def din(name, shape, dtype=F32):
        return nc.dram_tensor(name, shape, dtype, kind="ExternalInput")

    xT_d = din("xT", [C_IN, N], BF16)
    xsT_d = din("xsT", [C_IN, NS], BF16)
    ne8_d = din("ne8", [N, 8])
    iotab_d = din("iotab", [1, 256], BF16)
    idxq_d = din("idxq", [1, 32 * QS], BF16)
    iotak_d = din("iotak", [P, 2])
    ngc4_d = din("ngc4", [P, QT])
    eli0_d = din("eli0", [QS, 2], I32)
    eli1_d = din("eli1", [QS, 2], I32)
    dinv_d = din("dinvr", [1, NS])
    Wpe_d = din("Wpe", [C_IN, C], BF16)
    Wkp_d = din("Wkp", [C_IN, C], BF16)
    Wpev_d = din("Wpev", [C_IN, C], BF16)
    Wgcn_d = din("Wgcn", [C, C], BF16)
    Wq_d = din("Wq", [C, C], BF16)
    Wo_d = din("Wo", [C, C], BF16)
    Wm1_d = din("Wm1", [C, 2 * C], BF16)
    Wm2_d = din("Wm2", [2 * C, C], BF16)
    Wcm_d = din("Wcm", [C, C], BF16)
    Wsm_d = din("Wsm", [C, C], BF16)
    Wo1_d = din("Wo1", [2 * C, C], BF16)
    Wo2_d = din("Wo2", [C, 1], BF16)
    bpe_d = din("bpe", [P, 2])
    bq_d = din("bq", [P, 2])
    bgcn_d = din("bgcn", [P, 2])
    bocol_d = din("bocol", [P, 2])
    bn1g_d = din("bn1g", [P, 2]); bn1b_d = din("bn1b", [P, 2])
    bn2g_d = din("bn2g", [P, 2]); bn2b_d = din("bn2b", [P, 2])
    bm1_d = din("bm1", [P, 4])
    bm2_d = din("bm2", [P, 2])
    bn3g_d = din("bn3g", [P, 2]); bn3b_d = din("bn3b", [P, 2])
    bcm_d = din("bcm", [P, 2]); bsm_d = din("bsm", [P, 2])
    bo1_d = din("bo1", [P, 2])
    bnog_d = din("bnog", [P, 2]); bnob_d = din("bnob", [P, 2])
    bo2_d = din("bo2", [1, 1])
    t_d = din("tcol", [P, 1])

    out_d = nc.dram_tensor("out", [1, QS], F32, kind="ExternalOutput")
    dbg = {}

    def dout(name, shape, dtype=F32):
        if debug:
            dbg[name] = nc.dram_tensor("dbg_" + name, shape, dtype, kind="ExternalOutput")
            return dbg[name]
        return None

    dbg_pre1 = dout("pre1", [P, NS])
    dbg_pre2 = dout("pre2", [P, NS])
    dbg_z = dout("z", [P, NS])
    dbg_struct = dout("struct", [P, C])

    RG = [list(range(NC))]

    with tile.TileContext(nc) as tc:
        import contextlib
        with contextlib.ExitStack() as stk:
            perm = stk.enter_context(tc.tile_pool(name="perm", bufs=1))
            dram = stk.enter_context(tc.tile_pool(name="dram", bufs=1, space="DRAM"))
            colp = stk.enter_context(tc.tile_pool(name="colp", bufs=1))

            st12l = dram.tile([P, 8], F32, tag="st12l", name="st12l")
            st12g = dram.tile([P, 8], F32, tag="st12g", addr_space="Shared", name="st12g")
            st3l = dram.tile([P, 4], F32, tag="st3l", name="st3l")
            st3g = dram.tile([P, 4], F32, tag="st3g", addr_space="Shared", name="st3g")
            stol = dram.tile([P, 4], F32, tag="stol", name="stol")
            stog = dram.tile([P, 4], F32, tag="stog", addr_space="Shared", name="stog")
            NAG = 2
            zshk = [dram.tile([4 * P, C], BF16, tag=f"zsh{k}", name=f"zsh{k}")
                    for k in range(NAG)]
            zfgk = [dram.tile([NC * 4 * P, C], BF16, tag=f"zfg{k}",
                              addr_space="Shared", name=f"zfg{k}")
                    for k in range(NAG)]

            ident = perm.tile([P, P], F32, tag="ident", name="ident")
            make_identity(nc, ident[:])
            identw = perm.tile([P, P], F32R, tag="identw", name="identw")
            nc.vector.tensor_copy(identw[:], ident[:])
            identr = identw[:]
            identb = perm.tile([P, P], BF16, tag="identb", name="identb")
            nc.vector.tensor_copy(identb[:], ident[:])
            onesb = perm.tile([P, 1], BF16, tag="onesb", name="onesb")
            nc.vector.memset(onesb[:], 1.0)

            def loadw(d, K, Nw, tag):
                kt = K // P
                t = perm.tile([P, kt * Nw], BF16, tag=tag, name=tag)
                for k in range(kt):
                    nc.sync.dma_start(t[:, k * Nw:(k + 1) * Nw],
                                      d[k * P:(k + 1) * P, :])
                return t

            def loadc(d, cols, tag, dtype=F32):
                t = colp.tile([P, cols], dtype, tag=tag, name=tag)
                nc.sync.dma_start(t[:], d[:])
                return t

            Wpe_t = loadw(Wpe_d, C_IN, C, "Wpe_t")
            Wkp_t = loadw(Wkp_d, C_IN, C, "Wkp_t")
            Wpev_t = loadw(Wpev_d, C_IN, C, "Wpev_t")
            Wgcn_t = loadw(Wgcn_d, C, C, "Wgcn_t")
            Wq_t = loadw(Wq_d, C, C, "Wq_t")
            Wo_t = loadw(Wo_d, C, C, "Wo_t")
            Wm1_t = loadw(Wm1_d, C, 2 * C, "Wm1_t")
            Wm2_t = loadw(Wm2_d, 2 * C, C, "Wm2_t")
            Wcm_t = loadw(Wcm_d, C, C, "Wcm_t")
            Wsm_t = loadw(Wsm_d, C, C, "Wsm_t")
            Wo1_t = loadw(Wo1_d, 2 * C, C, "Wo1_t")
            Wo2_t = loadw(Wo2_d, C, 1, "Wo2_t")

            bpe = loadc(bpe_d, 2, "bpe")
            bq = loadc(bq_d, 2, "bq")
            bgcn = loadc(bgcn_d, 2, "bgcn")
            bocol = loadc(bocol_d, 2, "bocol")
            bn1g = loadc(bn1g_d, 2, "bn1g"); bn1b = loadc(bn1b_d, 2, "bn1b")
            bn2g = loadc(bn2g_d, 2, "bn2g"); bn2b = loadc(bn2b_d, 2, "bn2b")
            bm1c = loadc(bm1_d, 4, "bm1c")
            bm2c = loadc(bm2_d, 2, "bm2c")
            bn3g = loadc(bn3g_d, 2, "bn3g"); bn3b = loadc(bn3b_d, 2, "bn3b")
            bcm = loadc(bcm_d, 2, "bcm"); bsm = loadc(bsm_d, 2, "bsm")
            bo1c = loadc(bo1_d, 2, "bo1c")
            bnog = loadc(bnog_d, 2, "bnog"); bnob = loadc(bnob_d, 2, "bnob")
            tcol = loadc(t_d, 1, "tcol")
            bo2c = colp.tile([1, 1], F32, tag="bo2c", name="bo2c")
            nc.sync.dma_start(bo2c[:], bo2_d[:])
            ngct = loadc(ngc4_d, QT, "ngct")
            iotakt = loadc(iotak_d, 2, "iotakt")
            iotacol = [iotakt[:, k:k + 1] for k in range(2)]

            iotabs = colp.tile([1, 256], BF16, tag="iotabs", name="iotabs")
            nc.sync.dma_start(iotabs[:], iotab_d[:])
            iotaB0 = perm.tile([P, 256], BF16, tag="iotaB0", name="iotaB0")
            nc.gpsimd.partition_broadcast(iotaB0[:], iotabs[0:1, :])
            iotaB = [iotaB0] * 4

            hTs = [perm.tile([P, NS], BF16, tag=f"hTs{h}", name=f"hTs{h}") for h in range(2)]
            qT = [perm.tile([P, NS], BF16, tag=f"qT{h}", name=f"qT{h}") for h in range(2)]
            pre1 = [perm.tile([P, NS], F32R, tag=f"pre1_{h}", name=f"pre1_{h}") for h in range(2)]
            pre2 = [perm.tile([P, NS], F32R, tag=f"pre2_{h}", name=f"pre2_{h}") for h in range(2)]
            sq_scr = perm.tile([P, NS], F32, tag="sq_scr", name="sq_scr")

            def col(tag, n=1):
                return colp.tile([P, n], F32, tag=tag, name=tag)

            def bn_cols(s_ap, q_ap, g_ap, b_ap, n, scale_ap, bias_ap, mu_ap, v_ap):
                nc.vector.tensor_scalar_mul(mu_ap, s_ap, 1.0 / n)
                nc.vector.tensor_scalar_mul(v_ap, q_ap, 1.0 / n)
                nc.vector.scalar_tensor_tensor(scale_ap, mu_ap, 1.0, mu_ap, OP.mult, OP.mult)
                nc.vector.tensor_tensor(v_ap, v_ap, scale_ap, op=OP.subtract)
                nc.scalar.activation(v_ap, v_ap, AF.Sqrt, bias=epsc[:], scale=1.0)
                nc.vector.reciprocal(scale_ap, v_ap)
                nc.vector.tensor_tensor(scale_ap, scale_ap, g_ap, op=OP.mult)
                nc.vector.scalar_tensor_tensor(bias_ap, mu_ap, 1.0, scale_ap, OP.mult, OP.mult)
                nc.vector.tensor_tensor(bias_ap, b_ap, bias_ap, op=OP.subtract)

            mu_t = col("mu_t"); v_t = col("v_t")
            epsc = col("epsc")
            nc.vector.memset(epsc[:], EPS)
            negc = col("negc")
            nc.vector.memset(negc[:], -1.0)
            s1c = colp.tile([P, 4], F32, tag="s1c", name="s1c")
            q1c = colp.tile([P, 4], F32, tag="q1c", name="q1c")
            s2c = colp.tile([P, 4], F32, tag="s2c", name="s2c")
            q2c = colp.tile([P, 4], F32, tag="q2c", name="q2c")
            s3c = colp.tile([P, 4], F32, tag="s3c", name="s3c")
            q3c = colp.tile([P, 4], F32, tag="q3c", name="q3c")

            dinvr = colp.tile([1, NS], F32, tag="dinvr", name="dinvr")
            nc.sync.dma_start(dinvr[:], dinv_d[:])

            # ======== Phase 1: xT load, hT/kT/hTs/qT ========
            with tc.tile_pool(name="xp", bufs=1) as pool_x:
                dinvB = pool_x.tile([P, NS], F32, tag="dinvB", name="dinvB")
                dinv2B = pool_x.tile([P, NS], F32, tag="dinv2B", name="dinv2B")
                nc.gpsimd.partition_broadcast(dinvB[:], dinvr[:])
                nc.vector.tensor_tensor(dinv2B[:], dinvB[:], dinvB[:], op=OP.mult)
                xT = pool_x.tile([P, N], BF16, tag="xT", name="xT")
                for q4 in range(16):
                    nc.sync.dma_start(xT[:, q4 * 512:(q4 + 1) * 512],
                                      xT_d[:, q4 * 512:(q4 + 1) * 512])
                xsT = pool_x.tile([P, NS], BF16, tag="xsT", name="xsT")
                nc.sync.dma_start(xsT[:], xsT_d[:])

                with tc.tile_pool(name="kp", bufs=1) as pool_k:
                    kT = [pool_k.tile([P, N], BF16, tag=f"kT{h}", name=f"kT{h}")
                          for h in range(2)]
                    with tc.tile_pool(name="hp", bufs=1) as pool_h:
                        hT = [pool_h.tile([P, N], BF16, tag=f"hT{h}", name=f"hT{h}")
                              for h in range(2)]
                        with tc.tile_pool(name="psh", bufs=4, space="PSUM") as psh:
                            for ch in range(N // 512):
                                for m in range(2):
                                    ps = psh.tile([P, 512], F32, space="PSUM",
                                                  tag="ps", name="ps")
                                    nc.tensor.matmul(ps[:],
                                                     lhsT=Wpe_t[:, m * P:(m + 1) * P],
                                                     rhs=xT[:, ch * 512:(ch + 1) * 512],
                                                     start=True, stop=True)
                                    osl = hT[m][:, ch * 512:(ch + 1) * 512]
                                    if (ch + m) % 2 == 0:
                                        nc.vector.tensor_scalar(osl, ps[:],
                                                                bpe[:, m:m + 1],
                                                                None, OP.add)
                                    else:
                                        nc.scalar.activation(osl, ps[:], AF.Identity,
                                                             bias=bpe[:, m:m + 1],
                                                             scale=1.0)
                            for ch in range(N // 512):
                                for m in range(2):
                                    ps = psh.tile([P, 512], F32, space="PSUM",
                                                  tag="ps", name="ps")
                                    nc.tensor.matmul(ps[:],
                                                     lhsT=Wkp_t[:, m * P:(m + 1) * P],
                                                     rhs=xT[:, ch * 512:(ch + 1) * 512],
                                                     start=True, stop=True)
                                    osl = kT[m][:, ch * 512:(ch + 1) * 512]
                                    if (ch + m) % 2 == 0:
                                        nc.vector.tensor_copy(osl, ps[:])
                                    else:
                                        nc.scalar.copy(osl, ps[:])
                            for m in range(2):
                                for ch in range(2):
                                    ps = psh.tile([P, 512], F32, space="PSUM",
                                                  tag="ps", name="ps")
                                    nc.tensor.matmul(ps[:],
                                                     lhsT=Wpe_t[:, m * P:(m + 1) * P],
                                                     rhs=xsT[:, ch * 512:(ch + 1) * 512],
                                                     start=True, stop=True)
                                    nc.vector.tensor_scalar(
                                        hTs[m][:, ch * 512:(ch + 1) * 512],
                                        ps[:], bpe[:, m:m + 1], None, OP.add)
                            for m in range(2):
                                for ch in range(2):
                                    ps = psh.tile([P, 512], F32, space="PSUM",
                                                  tag="ps", name="ps")
                                    for k in range(2):
                                        nc.tensor.matmul(
                                            ps[:],
                                            lhsT=Wq_t[:, k * C + m * P: k * C + (m + 1) * P],
                                            rhs=hTs[k][:, ch * 512:(ch + 1) * 512],
                                            start=(k == 0), stop=(k == 1))
                                    nc.vector.tensor_scalar(
                                        qT[m][:, ch * 512:(ch + 1) * 512],
                                        ps[:], bq[:, m:m + 1], None, OP.add)

                        # ======== Phase 2: GCN single pass ========
                        with tc.tile_pool(name="ohp", bufs=2) as ohp, \
                             tc.tile_pool(name="nbp", bufs=1) as nbp, \
                             tc.tile_pool(name="hwp", bufs=2) as hwp, \
                             tc.tile_pool(name="psb", bufs=2, space="PSUM") as psb, \
                             tc.tile_pool(name="psagg", bufs=1, space="PSUM") as psagg:
                            ne8 = nbp.tile([P, NT * 8], F32, tag="ne8", name="ne8")
                            nc.sync.dma_start(
                                ne8[:].rearrange("p (s j) -> p s j", j=8),
                                ne8_d[:].rearrange("(s p) j -> p s j", p=P))
                            aggps = [psagg.tile([P, NS], F32, space="PSUM",
                                                tag=f"aggps{m}", name=f"aggps{m}")
                                     for m in range(2)]
                            for s in range(NT):
                                oh = ohp.tile([P, NS], BF16, tag="oh", name="oh")
                                for jj in range(4):
                                    nc.vector.tensor_scalar(
                                        oh[:, jj * 256:(jj + 1) * 256],
                                        iotaB[jj][:],
                                        ne8[:, s * 8 + jj:s * 8 + jj + 1],
                                        ne8[:, s * 8 + 4 + jj:s * 8 + 4 + jj + 1],
                                        OP.is_equal, OP.mult)
                                hwps = psb.tile([P, C], F32, space="PSUM",
                                                tag="hwps", name="hwps")
                                for k in range(2):
                                    nc.tensor.matmul(hwps[:],
                                                     lhsT=hT[k][:, s * P:(s + 1) * P],
                                                     rhs=Wgcn_t[:, k * C:(k + 1) * C],
                                                     start=(k == 0), stop=(k == 1))
                                hws = hwp.tile([P, C], BF16, tag="hws", name="hws")
                                if s % 2 == 0:
                                    nc.vector.tensor_copy(hws[:], hwps[:])
                                else:
                                    nc.scalar.copy(hws[:], hwps[:])
                                for m in range(2):
                                    for ch in range(2):
                                        nc.tensor.matmul(
                                            aggps[m][:, ch * 512:(ch + 1) * 512],
                                            lhsT=hws[:, m * P:(m + 1) * P],
                                            rhs=oh[:, ch * 512:(ch + 1) * 512],
                                            start=(s == 0), stop=(s == NT - 1),
                                            skip_group_check=True)
                            with tc.tile_pool(name="hw2p", bufs=2) as hw2p:
                                for m in range(2):
                                    for ch in range(2):
                                        chs = slice(ch * 512, (ch + 1) * 512)
                                        hwtps = psb.tile([P, 512], F32, space="PSUM",
                                                         tag="hwps", name="hwps")
                                        for k in range(2):
                                            nc.tensor.matmul(
                                                hwtps[:],
                                                lhsT=Wgcn_t[:, k * C + m * P: k * C + (m + 1) * P],
                                                rhs=hTs[k][:, chs],
                                                start=(k == 0), stop=(k == 1))
                                        p1s = pre1[m][:, chs]
                                        p1f = pre1[m][:].bitcast(F32)[:, chs]
                                        nc.vector.scalar_tensor_tensor(
                                            p1s, aggps[m][:, chs], 1.0, dinvB[:, chs],
                                            OP.mult, OP.mult)
                                        hw2 = hw2p.tile([P, 512], F32, tag="hw2", name="hw2")
                                        nc.vector.scalar_tensor_tensor(
                                            hw2[:], hwtps[:], 1.0, dinv2B[:, chs],
                                            OP.mult, OP.mult)
                                        nc.vector.tensor_tensor(p1s, p1f, hw2[:], op=OP.add)
                                        nc.vector.scalar_tensor_tensor(
                                            p1s, p1f, bgcn[:, m:m + 1],
                                            hTs[m][:, chs],
                                            OP.add, OP.add,
                                            accum_out=s1c[:, 2 * m + ch: 2 * m + ch + 1])
                                        nc.vector.scalar_tensor_tensor(
                                            sq_scr[:, :512], p1f, 1.0, p1f, OP.mult, OP.mult,
                                            accum_out=q1c[:, 2 * m + ch: 2 * m + ch + 1])
                        if debug:
                            nc.sync.dma_start(dbg_pre1[:], pre1[0][:].bitcast(F32))
                    # pool_h closed (hT freed)

                    # ======== Phase 3: vT + attention ========
                    with tc.tile_pool(name="vp", bufs=1) as pool_v:
                        vT = pool_v.tile([P, NT * C], BF16, tag="vT", name="vT")
                        with tc.tile_pool(name="psv", bufs=4, space="PSUM") as psv:
                            for kt in range(NT):
                                ps = psv.tile([P, C], F32, space="PSUM", tag="ps",
                                              name="ps")
                                nc.tensor.matmul(ps[:],
                                                 lhsT=xT[:, kt * P:(kt + 1) * P],
                                                 rhs=Wpev_t[:],
                                                 start=True, stop=True)
                                osl = vT[:, kt * C:(kt + 1) * C]
                                if kt % 2 == 0:
                                    nc.vector.tensor_copy(osl, ps[:])
                                else:
                                    nc.scalar.copy(osl, ps[:])
                        with tc.tile_pool(name="att", bufs=2) as att, \
                             tc.tile_pool(name="att1", bufs=1) as att1, \
                             tc.tile_pool(name="psatt", bufs=2, space="PSUM") as psatt, \
                             tc.tile_pool(name="psav", bufs=1, space="PSUM") as psav:
                            for ch2 in range(2):
                                chs = slice(ch2 * 512, (ch2 + 1) * 512)
                                avps = [psav.tile([P, 512], F32, space="PSUM",
                                                  tag=f"avps{m}", name=f"avps{m}")
                                        for m in range(2)]
                                denps = psav.tile([1, 512], F32, space="PSUM",
                                                  tag="denps", name="denps")
                                for kt in range(NT):
                                    stps = psatt.tile([P, 512], F32, space="PSUM",
                                                      tag="stps", name="stps")
                                    for k in range(2):
                                        nc.tensor.matmul(stps[:],
                                                         lhsT=kT[k][:, kt * P:(kt + 1) * P],
                                                         rhs=qT[k][:, chs],
                                                         start=(k == 0), stop=(k == 1))
                                    expt = att.tile([P, 512], BF16, tag="expt", name="expt")
                                    nc.scalar.activation(expt[:], stps[:], AF.Exp,
                                                         bias=0.0, scale=1.0 / 16.0)
                                    for m in range(2):
                                        nc.tensor.matmul(
                                            avps[m][:],
                                            lhsT=vT[:, kt * C + m * P: kt * C + (m + 1) * P],
                                            rhs=expt[:],
                                            start=(kt == 0), stop=(kt == NT - 1),
                                            skip_group_check=True)
                                    nc.tensor.matmul(denps[:], lhsT=onesb[:], rhs=expt[:],
                                                     start=(kt == 0), stop=(kt == NT - 1),
                                                     skip_group_check=True)
                                dens = att1.tile([1, 512], F32, tag="dens", name="dens")
                                nc.vector.tensor_copy(dens[:], denps[:])
                                nc.vector.reciprocal(dens[:], dens[:])
                                rdenB = att1.tile([P, 512], F32, tag="rdenB", name="rdenB")
                                nc.gpsimd.partition_broadcast(rdenB[:], dens[:])
                                attno = [att1.tile([P, 512], BF16, tag=f"attno{m}",
                                                   name=f"attno{m}")
                                         for m in range(2)]
                                for m in range(2):
                                    nc.vector.scalar_tensor_tensor(
                                        attno[m][:], avps[m][:], 1.0, rdenB[:],
                                        OP.mult, OP.mult)
                                for m in range(2):
                                    ps = psatt.tile([P, 512], F32, space="PSUM",
                                                    tag="stps", name="stps")
                                    for k in range(2):
                                        nc.tensor.matmul(
                                            ps[:],
                                            lhsT=Wo_t[:, k * C + m * P: k * C + (m + 1) * P],
                                            rhs=attno[k][:],
                                            start=(k == 0), stop=(k == 1))
                                    p2s = pre2[m][:, chs]
                                    p2f = pre2[m][:].bitcast(F32)[:, chs]
                                    nc.vector.scalar_tensor_tensor(
                                        p2s, ps[:], bocol[:, m:m + 1],
                                        hTs[m][:, chs], OP.add, OP.add,
                                        accum_out=s2c[:, 2 * m + ch2: 2 * m + ch2 + 1])
                                    nc.vector.scalar_tensor_tensor(
                                        sq_scr[:, :512], p2f, 1.0, p2f, OP.mult, OP.mult,
                                        accum_out=q2c[:, 2 * m + ch2: 2 * m + ch2 + 1])
                        if debug:
                            nc.sync.dma_start(dbg_pre2[:], pre2[0][:].bitcast(F32))
                # pool_k closed
            # pool_x closed

            # ======== Phase 6 prep (fills the st12-AR stall) ========
            stp = stk.enter_context(tc.tile_pool(name="stp", bufs=1))
            selp = stk.enter_context(tc.tile_pool(name="selp", bufs=1))
            sml = stk.enter_context(tc.tile_pool(name="sml", bufs=2))
            e0t, e1t = [], []
            for i in range(QT):
                e0 = stp.tile([P, 2], I32, tag=f"e0_{i}", name=f"e0_{i}")
                e1 = stp.tile([P, 2], I32, tag=f"e1_{i}", name=f"e1_{i}")
                nc.sync.dma_start(e0[:], eli0_d[i * P:(i + 1) * P, :])
                nc.sync.dma_start(e1[:], eli1_d[i * P:(i + 1) * P, :])
                e0t.append(e0); e1t.append(e1)
            idxqf = [stp.tile([1, 32 * P], BF16, tag=f"idxqf{i}", name=f"idxqf{i}")
                     for i in range(QT)]
            for i in range(QT):
                nc.sync.dma_start(idxqf[i][:], idxq_d[0:1, i * 32 * P:(i + 1) * 32 * P])
            jorder = [k + 4 * t for k in range(CH) for t in range(8)]

            def build_sel(i, slot):
                sels = []
                for dj in range(2):
                    j = jorder[2 * slot + dj]
                    idxB = sml.tile([P, P], BF16, tag="idxB", name="idxB")
                    nc.gpsimd.partition_broadcast(
                        idxB[:], idxqf[i][0:1, j * P:(j + 1) * P])
                    for k in range(2):
                        sel = selp.tile([P, P], BF16, tag=f"sel_{slot % 4}_{dj}_{k}_{i}",
                                        name=f"sel_{slot}_{dj}_{k}_{i}")
                        eng = nc.vector if k == 0 else nc.gpsimd
                        eng.tensor_scalar(sel[:], idxB[:], iotacol[k],
                                          None, OP.is_equal)
                        sels.append(sel)
                return sels

            sel_next = [[build_sel(i, slot) for slot in range(4)]
                        for i in range(QT)]

            # ======== Phase 4: BN1/BN2 -> zb, MLP, z transpose + chunked AG ========
            stat = colp.tile([P, 8], F32, tag="stat", name="stat")
            for m in range(2):
                nc.vector.tensor_tensor(stat[:, m:m + 1], s1c[:, 2 * m:2 * m + 1],
                                        s1c[:, 2 * m + 1:2 * m + 2], op=OP.add)
                nc.vector.tensor_tensor(stat[:, 2 + m:3 + m], q1c[:, 2 * m:2 * m + 1],
                                        q1c[:, 2 * m + 1:2 * m + 2], op=OP.add)
                nc.vector.tensor_tensor(stat[:, 4 + m:5 + m], s2c[:, 2 * m:2 * m + 1],
                                        s2c[:, 2 * m + 1:2 * m + 2], op=OP.add)
                nc.vector.tensor_tensor(stat[:, 6 + m:7 + m], q2c[:, 2 * m:2 * m + 1],
                                        q2c[:, 2 * m + 1:2 * m + 2], op=OP.add)
            nc.sync.dma_start(st12l[:], stat[:])
            nc.gpsimd.collective_compute("AllReduce", OP.add, replica_groups=RG,
                                         ins=[st12l[:].opt()], outs=[st12g[:].opt()])
            statg = colp.tile([P, 8], F32, tag="statg", name="statg")
            nc.sync.dma_start(statg[:], st12g[:])
            sc1 = colp.tile([P, 2], F32, tag="sc1", name="sc1")
            bi1 = colp.tile([P, 2], F32, tag="bi1", name="bi1")
            sc2 = colp.tile([P, 2], F32, tag="sc2", name="sc2")
            bi2 = colp.tile([P, 2], F32, tag="bi2", name="bi2")
            for m in range(2):
                bn_cols(statg[:, m:m + 1], statg[:, 2 + m:3 + m],
                        bn1g[:, m:m + 1], bn1b[:, m:m + 1], N,
                        sc1[:, m:m + 1], bi1[:, m:m + 1], mu_t[:], v_t[:])
                bn_cols(statg[:, 4 + m:5 + m], statg[:, 6 + m:7 + m],
                        bn2g[:, m:m + 1], bn2b[:, m:m + 1], N,
                        sc2[:, m:m + 1], bi2[:, m:m + 1], mu_t[:], v_t[:])
            b12 = colp.tile([P, 2], F32, tag="b12", name="b12")
            nc.vector.tensor_tensor(b12[:], bi1[:], bi2[:], op=OP.add)
            # zb (bf16) reuses qT tiles
            zb = qT
            for m in range(2):
                p1f = pre1[m][:].bitcast(F32)
                p2f = pre2[m][:].bitcast(F32)
                nc.vector.tensor_scalar(zb[m][:], p1f, sc1[:, m:m + 1], b12[:, m:m + 1],
                                        OP.mult, OP.add)
                nc.vector.scalar_tensor_tensor(zb[m][:], p2f, sc2[:, m:m + 1], zb[m][:],
                                               OP.mult, OP.add)
            if debug:
                zdbg = colp.tile([P, NS], F32, tag="zdbg", name="zdbg")
                nc.vector.tensor_copy(zdbg[:], zb[0][:])
                nc.sync.dma_start(dbg_z[:], zdbg[:])
            m1T = [perm.tile([P, NS], BF16, tag=t, name=f"m1T{mi}")
                   for mi, t in enumerate(("hTs0", "hTs1", "pre2_0", "pre2_1"))]
            with tc.tile_pool(name="psm", bufs=4, space="PSUM") as psm:
                for mi in range(4):
                    for ch in range(2):
                        ps = psm.tile([P, 512], F32, space="PSUM", tag="ps", name="ps")
                        for k in range(2):
                            nc.tensor.matmul(
                                ps[:],
                                lhsT=Wm1_t[:, k * 2 * C + mi * P: k * 2 * C + (mi + 1) * P],
                                rhs=zb[k][:, ch * 512:(ch + 1) * 512],
                                start=(k == 0), stop=(k == 1))
                        nc.scalar.activation(m1T[mi][:, ch * 512:(ch + 1) * 512], ps[:],
                                             AF.Relu, bias=bm1c[:, mi:mi + 1], scale=1.0)
                for m in range(2):
                    for ch in range(2):
                        chs = slice(ch * 512, (ch + 1) * 512)
                        ps = psm.tile([P, 512], F32, space="PSUM", tag="ps", name="ps")
                        for k4 in range(4):
                            nc.tensor.matmul(
                                ps[:],
                                lhsT=Wm2_t[:, k4 * C + m * P: k4 * C + (m + 1) * P],
                                rhs=m1T[k4][:, chs],
                                start=(k4 == 0), stop=(k4 == 3))
                        p1s = pre1[m][:, chs]
                        p1f = pre1[m][:].bitcast(F32)[:, chs]
                        nc.vector.scalar_tensor_tensor(
                            p1s, ps[:], bm2c[:, m:m + 1], zb[m][:, chs], OP.add, OP.add,
                            accum_out=s3c[:, 2 * m + ch: 2 * m + ch + 1])
                        nc.vector.scalar_tensor_tensor(
                            sq_scr[:, :512], p1f, 1.0, p1f, OP.mult, OP.mult,
                            accum_out=q3c[:, 2 * m + ch: 2 * m + ch + 1])
            # stat3 -> AR (issued before the AGs; result consumed mid phase 6)
            stat3 = colp.tile([P, 4], F32, tag="stat3", name="stat3")
            for m in range(2):
                nc.vector.tensor_tensor(stat3[:, m:m + 1], s3c[:, 2 * m:2 * m + 1],
                                        s3c[:, 2 * m + 1:2 * m + 2], op=OP.add)
                nc.vector.tensor_tensor(stat3[:, 2 + m:3 + m], q3c[:, 2 * m:2 * m + 1],
                                        q3c[:, 2 * m + 1:2 * m + 2], op=OP.add)
            nc.sync.dma_start(st3l[:], stat3[:])
            nc.gpsimd.collective_compute("AllReduce", OP.add, replica_groups=RG,
                                         ins=[st3l[:].opt()], outs=[st3g[:].opt()])
            # z transpose per chunk + AG
            with tc.tile_pool(name="psz", bufs=2, space="PSUM") as psz, \
                 tc.tile_pool(name="znp", bufs=1) as znp:
                znsh = znp.tile([P, 8 * C], BF16, tag="znsh", name="znsh")
                for k in range(2):
                    for h in range(4):
                        nt = 4 * k + h
                        for m in range(2):
                            ps = psz.tile([P, P], F32R, space="PSUM", tag="ps", name="ps")
                            nc.tensor.matmul(ps[:], lhsT=pre1[m][:, nt * P:(nt + 1) * P],
                                             rhs=identr, is_transpose=True)
                            osl = znsh[:, nt * C + m * P: nt * C + (m + 1) * P]
                            if m == 0:
                                nc.vector.tensor_copy(osl, ps[:].bitcast(F32))
                            else:
                                nc.scalar.copy(osl, ps[:].bitcast(F32))
                    nc.sync.dma_start(
                        zshk[k][:].rearrange("(h p) c -> p h c", p=P),
                        znsh[:, 4 * k * C:(4 * k + 4) * C].rearrange(
                            "p (h c) -> p h c", c=C))
                    nc.gpsimd.collective_compute(
                        "AllGather", OP.bypass, replica_groups=RG,
                        ins=[zshk[k][:].opt()], outs=[zfgk[k][:].opt()])

            # ======== Phase 5: structure + content + predictor ========
            with tc.tile_pool(name="zn", bufs=1) as pool_zn, \
                 tc.tile_pool(name="pss", bufs=2, space="PSUM") as pss:
                # sc3/bi3 from the AR (arrives during AGs)
                stat3g = colp.tile([P, 4], F32, tag="stat3g", name="stat3g")
                nc.sync.dma_start(stat3g[:], st3g[:])
                sc3 = colp.tile([P, 2], F32, tag="sc3", name="sc3")
                bi3 = colp.tile([P, 2], F32, tag="bi3", name="bi3")
                for m in range(2):
                    bn_cols(stat3g[:, m:m + 1], stat3g[:, 2 + m:3 + m],
                            bn3g[:, m:m + 1], bn3b[:, m:m + 1], N,
                            sc3[:, m:m + 1], bi3[:, m:m + 1], mu_t[:], v_t[:])
                # rows [1, C] via PE transpose + per-m DMAs, then broadcast
                sc3row = colp.tile([1, C], F32, tag="sc3row", name="sc3row")
                bi3row = colp.tile([1, C], F32, tag="bi3row", name="bi3row")
                with tc.tile_pool(name="psr", bufs=1, space="PSUM") as psr:
                    for src, row in ((sc3, sc3row), (bi3, bi3row)):
                        srcr = sml.tile([P, 2], F32R, tag="srcr", name="srcr")
                        nc.vector.tensor_copy(srcr[:], src[:])
                        psT = psr.tile([2, P], F32R, space="PSUM", tag="psT", name="psT")
                        nc.tensor.matmul(psT[:], lhsT=srcr[:],
                                         rhs=identr, is_transpose=True)
                        s2 = sml.tile([2, P], F32, tag="s2row", name="s2row")
                        nc.vector.tensor_copy(s2[:], psT[:].bitcast(F32))
                        for m in range(2):
                            nc.sync.dma_start(row[0:1, m * P:(m + 1) * P],
                                              s2[m:m + 1, :])
                sc3B = stp.tile([P, C], F32, tag="sc3B", name="sc3B")
                bi3B = stp.tile([P, C], F32, tag="bi3B", name="bi3B")
                nc.gpsimd.partition_broadcast(sc3B[:], sc3row[:])
                nc.gpsimd.partition_broadcast(bi3B[:], bi3row[:])
                sc3rep = stp.tile([P, 16 * C], BF16, tag="sc3rep", name="sc3rep")
                nc.vector.tensor_copy(sc3rep[:, :C], sc3B[:])
                w = C
                while w < 16 * C:
                    nc.vector.tensor_copy(sc3rep[:, w:2 * w], sc3rep[:, :w])
                    w *= 2

                z_node = pool_zn.tile([P, NT * C], BF16, tag="z_node", name="z_node")
                g0a = [stp.tile([P, C], BF16, tag=f"g0_{i}", name=f"g0_{i}")
                       for i in range(QT)]
                g1a = [stp.tile([P, C], BF16, tag=f"g1_{i}", name=f"g1_{i}")
                       for i in range(QT)]
                num_acc = [stp.tile([P, C], F32, tag=f"numa{i}", name=f"numa{i}")
                           for i in range(QT)]
                den_acc = [stp.tile([P, C], F32, tag=f"dena{i}", name=f"dena{i}")
                           for i in range(QT)]

                for kc in range(CH):
                    # z_node columns for this quarter (waits on AG kc//2)
                    kg, kh = kc // 2, kc % 2
                    for cc in range(NC):
                        zsl = z_node[:, (cc * 8 + 2 * kc) * C:(cc * 8 + 2 * kc + 2) * C]
                        nc.sync.dma_start(
                            zsl.rearrange("p (h c) -> p h c", c=C),
                            zfgk[kg][cc * 4 * P + kh * 2 * P:
                                     cc * 4 * P + (kh + 1) * 2 * P, :]
                            .rearrange("(h p) c -> p h c", p=P))
                        nc.vector.tensor_tensor(zsl, zsl, sc3rep[:, :2 * C],
                                                op=OP.mult)
                    # content gathers (once per AG tile, on its first quarter)
                    if kh == 0:
                        for i in range(QT):
                            nc.gpsimd.indirect_dma_start(
                                out=g0a[i][:], out_offset=None, in_=zfgk[kg][:],
                                in_offset=bass.IndirectOffsetOnAxis(
                                    ap=e0t[i][:, kg:kg + 1], axis=0),
                                bounds_check=4 * P * NC - 1, oob_is_err=False)
                            nc.gpsimd.indirect_dma_start(
                                out=g1a[i][:], out_offset=None, in_=zfgk[kg][:],
                                in_offset=bass.IndirectOffsetOnAxis(
                                    ap=e1t[i][:, kg:kg + 1], axis=0),
                                bounds_check=4 * P * NC - 1, oob_is_err=False)
                    sel_cur, sel_next = sel_next, None
                    for i in range(QT):
                        zgk = sml.tile([P, 8 * C], BF16, tag="zgk", name="zgk")
                        for sl in range(4):
                            slot = 4 * kc + sl
                            zgps = pss.tile([P, 2 * C], F32, space="PSUM", tag="zgps",
                                            name="zgps", bufs=2)
                            for dj in range(2):
                                j = jorder[2 * slot + dj]
                                for k in range(2):
                                    sel = sel_cur[i][sl][2 * dj + k]
                                    nc.tensor.matmul(
                                        zgps[:, dj * C:(dj + 1) * C],
                                        lhsT=sel[:],
                                        rhs=z_node[:, (2 * j + k) * C:(2 * j + k + 1) * C],
                                        start=(k == 0), stop=(k == 1),
                                        skip_group_check=True)
                            osl = zgk[:, 2 * sl * C:(2 * sl + 2) * C]
                            if sl % 2 == 0:
                                nc.vector.tensor_copy(osl, zgps[:])
                            else:
                                nc.scalar.copy(osl, zgps[:])
                        ebk = sml.tile([P, 8 * C], BF16, tag="ebk", name="ebk")
                        nc.scalar.activation(ebk[:], zgk[:], AF.Exp,
                                             bias=0.0, scale=tcol[:, :1])
                        nc.vector.tensor_tensor(zgk[:], ebk[:], zgk[:], op=OP.mult)
                        nc.scalar.activation(ebk[:], ebk[:], AF.Identity,
                                             bias=negc[:], scale=1.0)
                        w = 4 * C
                        while w >= C:
                            nc.gpsimd.tensor_tensor(ebk[:, :w], ebk[:, :w],
                                                    ebk[:, w:2 * w], op=OP.add)
                            nc.vector.tensor_tensor(zgk[:, :w], zgk[:, :w],
                                                    zgk[:, w:2 * w], op=OP.add)
                            w //= 2
                        if kc == 0:
                            nc.vector.tensor_copy(num_acc[i][:], zgk[:, :C])
                            nc.scalar.copy(den_acc[i][:], ebk[:, :C])
                        else:
                            nc.vector.tensor_tensor(num_acc[i][:], num_acc[i][:],
                                                    zgk[:, :C], op=OP.add)
                            nc.gpsimd.tensor_tensor(den_acc[i][:], den_acc[i][:],
                                                    ebk[:, :C], op=OP.add)
                    if kc + 1 < CH:
                        sel_next = [[build_sel(i, 4 * (kc + 1) + sl) for sl in range(4)]
                                    for i in range(QT)]

                structT = [stp.tile([P, QS], BF16, tag=f"structT{m}", name=f"structT{m}")
                           for m in range(2)]
                contT = [stp.tile([P, QS], BF16, tag=f"contT{m}", name=f"contT{m}")
                         for m in range(2)]
                for i in range(QT):
                    den = sml.tile([P, C], F32, tag="den", name="den")
                    nc.vector.tensor_scalar(den[:], den_acc[i][:], ngct[:, i:i + 1],
                                            1e-12, OP.add, OP.max)
                    nc.vector.reciprocal(den[:], den[:])
                    st_i = sml.tile([P, C], BF16, tag="st_i", name="st_i")
                    nc.vector.tensor_tensor(st_i[:], num_acc[i][:], den[:], op=OP.mult)
                    gate = sml.tile([P, 1], F32, tag="gate", name="gate")
                    nc.vector.tensor_scalar(gate[:], ngct[:, i:i + 1], 1.0, None, OP.min)
                    bi3g = sml.tile([P, C], F32, tag="bi3g", name="bi3g")
                    nc.vector.tensor_scalar(bi3g[:], bi3B[:], gate[:, :1], None, OP.mult)
                    nc.vector.tensor_tensor(st_i[:], st_i[:], bi3g[:], op=OP.add)
                    if debug and i == 0:
                        stf = sml.tile([P, C], F32, tag="stf", name="stf")
                        nc.vector.tensor_copy(stf[:], st_i[:])
                        nc.sync.dma_start(dbg_struct[:], stf[:])
                    zc0 = sml.tile([P, C], F32, tag="zc0", name="zc0")
                    zc1 = sml.tile([P, C], F32, tag="zc1", name="zc1")
                    nc.vector.tensor_tensor(zc0[:], g0a[i][:], sc3B[:], op=OP.mult)
                    nc.vector.tensor_tensor(zc0[:], zc0[:], bi3B[:], op=OP.add)
                    nc.vector.tensor_tensor(zc1[:], g1a[i][:], sc3B[:], op=OP.mult)
                    nc.vector.tensor_tensor(zc1[:], zc1[:], bi3B[:], op=OP.add)
                    ct_i = sml.tile([P, C], BF16, tag="ct_i", name="ct_i")
                    nc.vector.tensor_tensor(ct_i[:], zc0[:], zc1[:], op=OP.mult)
                    for m in range(2):
                        ps = pss.tile([P, P], BF16, space="PSUM", tag="pstr2", name="pstr2")
                        nc.tensor.matmul(ps[:], lhsT=st_i[:, m * P:(m + 1) * P],
                                         rhs=identb[:], is_transpose=True)
                        nc.vector.tensor_copy(structT[m][:, i * P:(i + 1) * P], ps[:])
                        ps2 = pss.tile([P, P], BF16, space="PSUM", tag="pstr2", name="pstr2")
                        nc.tensor.matmul(ps2[:], lhsT=ct_i[:, m * P:(m + 1) * P],
                                         rhs=identb[:], is_transpose=True)
                        nc.scalar.copy(contT[m][:, i * P:(i + 1) * P], ps2[:])

                soc = colp.tile([P, 2], F32, tag="soc", name="soc")
                qoc = colp.tile([P, 2], F32, tag="qoc", name="qoc")
                hcat = [stp.tile([P, QS], BF16, tag=tg, name=f"hcat{mi}")
                        for mi, tg in enumerate(("contT0", "contT1", "structT0",
                                                 "structT1"))]
                ps_cm = [pss.tile([P, QS], F32, space="PSUM", tag="pspred", name="pspred")
                         for _ in range(2)]
                for m in range(2):
                    for k in range(2):
                        nc.tensor.matmul(ps_cm[m][:],
                                         lhsT=Wcm_t[:, k * C + m * P: k * C + (m + 1) * P],
                                         rhs=contT[k][:], start=(k == 0), stop=(k == 1))
                for m in range(2):
                    nc.vector.tensor_scalar(hcat[m][:], ps_cm[m][:], bcm[:, m:m + 1],
                                            None, OP.add)
                ps_sm = [pss.tile([P, QS], F32, space="PSUM", tag="pspred", name="pspred")
                         for _ in range(2)]
                for m in range(2):
                    for k in range(2):
                        nc.tensor.matmul(ps_sm[m][:],
                                         lhsT=Wsm_t[:, k * C + m * P: k * C + (m + 1) * P],
                                         rhs=structT[k][:], start=(k == 0), stop=(k == 1))
                for m in range(2):
                    nc.vector.tensor_scalar(hcat[2 + m][:], ps_sm[m][:], bsm[:, m:m + 1],
                                            None, OP.add)
                preo = [stp.tile([P, QS], F32R, tag=f"preo{m}", name=f"preo{m}")
                        for m in range(2)]
                for m in range(2):
                    ps = pss.tile([P, QS], F32, space="PSUM", tag="pspred", name="pspred")
                    for k4 in range(4):
                        nc.tensor.matmul(ps[:],
                                         lhsT=Wo1_t[:, k4 * C + m * P: k4 * C + (m + 1) * P],
                                         rhs=hcat[k4][:], start=(k4 == 0), stop=(k4 == 3))
                    nc.vector.tensor_scalar(preo[m][:], ps[:], bo1c[:, m:m + 1], 0.0,
                                            OP.add, OP.add, accum_out=soc[:, m:m + 1])
                    pof = preo[m][:].bitcast(F32)
                    nc.vector.scalar_tensor_tensor(
                        sq_scr[:, :QS], pof, 1.0, pof, OP.mult, OP.mult,
                        accum_out=qoc[:, m:m + 1])
                stato = colp.tile([P, 4], F32, tag="stato", name="stato")
                nc.vector.tensor_copy(stato[:, 0:2], soc[:])
                nc.vector.tensor_copy(stato[:, 2:4], qoc[:])
                nc.sync.dma_start(stol[:], stato[:])
                nc.gpsimd.collective_compute("AllReduce", OP.add, replica_groups=RG,
                                             ins=[stol[:].opt()], outs=[stog[:].opt()])
                statog = colp.tile([P, 4], F32, tag="statog", name="statog")
                nc.sync.dma_start(statog[:], stog[:])
                sco = colp.tile([P, 2], F32, tag="sco", name="sco")
                bio = colp.tile([P, 2], F32, tag="bio", name="bio")
                for m in range(2):
                    bn_cols(statog[:, m:m + 1], statog[:, 2 + m:3 + m],
                            bnog[:, m:m + 1], bnob[:, m:m + 1], Q,
                            sco[:, m:m + 1], bio[:, m:m + 1], mu_t[:], v_t[:])
                hmid = [stp.tile([P, QS], BF16, tag=f"hmid{m}", name=f"hmid{m}")
                        for m in range(2)]
                for m in range(2):
                    nc.scalar.activation(hmid[m][:], preo[m][:].bitcast(F32), AF.Relu,
                                         bias=bio[:, m:m + 1], scale=sco[:, m:m + 1])
                outps = pss.tile([1, QS], F32, space="PSUM", tag="outps", name="outps",
                                 bufs=1)
                for k in range(2):
                    nc.tensor.matmul(outps[:], lhsT=Wo2_t[:, k:k + 1], rhs=hmid[k][:],
                                     start=(k == 0), stop=(k == 1))
                outsb = stp.tile([1, QS], F32, tag="outsb", name="outsb")
                nc.vector.tensor_scalar(outsb[:], outps[:], bo2c[:, :1], None, OP.add)
                nc.sync.dma_start(out_d[:], outsb[:])

    nc.compile()
    return nc, dbg


def _prep_inputs(x, edge_index, edge_weight, edge_label_index, nbr, weights):
    import ml_dtypes
    BFNP = ml_dtypes.bfloat16
    x = np.asarray(x, np.float32)
    nbr = np.asarray(nbr)
    ew = np.asarray(edge_weight, np.float32)
    eli = np.asarray(edge_label_index)
    src = np.asarray(edge_index[0])
    assert np.array_equal(src, np.repeat(np.arange(N, dtype=src.dtype), D)), \
        "edge_index[0] structure mismatch"
    assert np.array_equal(np.asarray(edge_index[1]), nbr.reshape(-1)), \
        "edge_index[1] != nbr.flatten()"
    block = N // D
    assert ((nbr // block) == np.arange(D)[None, :]).all(), "nbr not block-structured"

    nbrf = nbr.astype(np.float32)
    ew2 = ew.reshape(N, D).astype(np.float32)
    nbr_i = nbr.astype(np.int64)
    w = {k: np.asarray(v, np.float32) for k, v in weights.items()}

    # ---- host folds ----
    mu0 = x.mean(0)
    var0 = ((x - mu0) ** 2).mean(0)
    scale0 = w["pe_gamma"] / np.sqrt(var0 + EPS)
    bias0 = w["pe_beta"] - mu0 * scale0
    Wpe_eff = scale0[:, None] * w["W_pe"]
    bpe_eff = w["b_pe"] + bias0 @ w["W_pe"]
    Wkp = Wpe_eff @ w["Wk"]
    Wpev = Wpe_eff @ w["Wv"]
    rcol = bpe_eff @ w["Wv"] + w["bv"]
    bocol_h = rcol @ w["Wo"] + w["bo"]
    dst = np.asarray(edge_index[1]).astype(np.int64)
    deg = np.bincount(dst, weights=ew.astype(np.float64), minlength=N).astype(
        np.float32) + 1.0
    dinv = (1.0 / np.sqrt(deg)).astype(np.float32)

    def colsplit(v, k):
        return np.ascontiguousarray(v.reshape(k, P).T.astype(np.float32))

    def bfw(a):
        return np.ascontiguousarray(np.asarray(a, BFNP))

    common = {
        "xT": bfw(x.T),
        "Wpe": bfw(Wpe_eff), "Wkp": bfw(Wkp), "Wpev": bfw(Wpev),
        "Wgcn": bfw(w["W_gcn"]), "Wq": bfw(w["Wq"]), "Wo": bfw(w["Wo"]),
        "Wm1": bfw(w["W_m1"]), "Wm2": bfw(w["W_m2"]),
        "Wcm": bfw(w["W_cm"]), "Wsm": bfw(w["W_sm"]),
        "Wo1": bfw(w["W_o1"]), "Wo2": bfw(w["W_o2"]),
        "bpe": colsplit(bpe_eff, 2), "bq": colsplit(w["bq"], 2),
        "bgcn": colsplit(w["b_gcn"], 2), "bocol": colsplit(bocol_h, 2),
        "bn1g": colsplit(w["bn1_g"], 2), "bn1b": colsplit(w["bn1_b"], 2),
        "bn2g": colsplit(w["bn2_g"], 2), "bn2b": colsplit(w["bn2_b"], 2),
        "bm1": colsplit(w["b_m1"], 4), "bm2": colsplit(w["b_m2"], 2),
        "bn3g": colsplit(w["bn3_g"], 2), "bn3b": colsplit(w["bn3_b"], 2),
        "bcm": colsplit(w["b_cm"], 2), "bsm": colsplit(w["b_sm"], 2),
        "bo1": colsplit(w["b_o1"], 2),
        "bnog": colsplit(w["bno_g"], 2), "bnob": colsplit(w["bno_b"], 2),
        "bo2": w["b_o2"].reshape(1, 1),
        "tcol": np.full((P, 1), float(np.asarray(weights["t"])), np.float32),
        "iotak": np.stack([np.arange(P), np.arange(P) + P], 1).astype(np.float32),
    }
    in_maps = []
    for c in range(NC):
        blocks = np.arange(4 * c, 4 * c + 4)
        iotab = (blocks[:, None] * block + np.arange(block)[None, :]).astype(np.float32)
        m = dict(common)
        m["xsT"] = bfw(x[c * NS:(c + 1) * NS, :].T)
        m["ne8"] = np.ascontiguousarray(np.concatenate(
            [nbrf[:, blocks] - (blocks * block)[None, :],
             ew2[:, blocks] * dinv[:, None]], axis=1).astype(np.float32))
        m["iotab"] = np.ascontiguousarray(
            np.arange(block, dtype=np.float32).reshape(1, block).astype(BFNP))
        m["dinvr"] = np.ascontiguousarray(dinv[c * NS:(c + 1) * NS].reshape(1, NS))
        e0 = eli[0, c * QS:(c + 1) * QS].astype(np.int64)
        e1 = eli[1, c * QS:(c + 1) * QS].astype(np.int64)

        def chunkidx(e):
            local = e % NS
            kchunk = local // 512
            row = (e // NS) * 512 + (local % 512)
            cols = np.full((QS, 2), OOB, np.int32)
            cols[np.arange(QS), kchunk] = row.astype(np.int32)
            return np.ascontiguousarray(cols)

        m["eli0"] = chunkidx(e0)
        m["eli1"] = chunkidx(e1)
        t_nbr = nbr_i[e0]              # [QS, 32]
        s_nbr = nbr_i[e1]
        mask = (s_nbr == t_nbr)
        cnt = mask.sum(1).astype(np.float32)
        idxp = np.where(mask, t_nbr - (np.arange(D) * block)[None, :], int(BIG))
        idxq = idxp.reshape(QT, P, D).transpose(0, 2, 1).reshape(1, -1)
        m["idxq"] = np.ascontiguousarray(idxq.astype(BFNP))
        m["ngc4"] = np.ascontiguousarray(cnt.reshape(QT, P).T.astype(np.float32))
        in_maps.append(m)
    return in_maps


def kernel(x, edge_index, edge_weight, edge_label_index, nbr,
           pe_gamma, pe_beta, W_pe, b_pe, W_gcn, b_gcn, bn1_g, bn1_b,
           Wq, bq, Wk, bk, Wv, bv, Wo, bo, bn2_g, bn2_b,
           W_m1, b_m1, W_m2, b_m2, bn3_g, bn3_b,
           W_cm, b_cm, W_sm, b_sm,
           W_o1, b_o1, bno_g, bno_b, W_o2, b_o2, t, _debug=False, _results=None):
    from concourse.bass_utils import run_bass_kernel_spmd

    key = "dbg" if _debug else "main"
    if key not in _CACHE:
        _CACHE[key] = _build(debug=_debug)
    nc, dbg = _CACHE[key]

    weights = dict(pe_gamma=pe_gamma, pe_beta=pe_beta, W_pe=W_pe, b_pe=b_pe,
                   W_gcn=W_gcn, b_gcn=b_gcn, bn1_g=bn1_g, bn1_b=bn1_b,
                   Wq=Wq, bq=bq, Wk=Wk, bk=bk, Wv=Wv, bv=bv, Wo=Wo, bo=bo,
                   bn2_g=bn2_g, bn2_b=bn2_b, W_m1=W_m1, b_m1=b_m1,
                   W_m2=W_m2, b_m2=b_m2, bn3_g=bn3_g, bn3_b=bn3_b,
                   W_cm=W_cm, b_cm=b_cm, W_sm=W_sm, b_sm=b_sm,
                   W_o1=W_o1, b_o1=b_o1, bno_g=bno_g, bno_b=bno_b,
                   W_o2=W_o2, b_o2=b_o2, t=t)
    in_maps = _prep_inputs(x, edge_index, edge_weight, edge_label_index, nbr, weights)
    r = run_bass_kernel_spmd(nc, in_maps, core_ids=list(range(NC)))
    if _results is not None:
        _results.extend(r.results)
    out = np.concatenate([r.results[c]["out"][0] for c in range(NC)])
    return out.reshape(Q, 1).astype(np.float32)

